# revision 13
# baseline (speedup 1.0000x reference)
"""GCN-LSTM regressor as a Bass/Tile kernel for 8 Trainium2 NeuronCores.

Math restructuring (exact, up to fp reassociation):
  The reference GCN is rank-2 in disguise:
    m  = A_hat @ x_bt          (over lines)         (B,T,L)
    h1 = relu(m[...,None] * W1) ;  xw2 = h1 @ W2
       = m+ * u+  +  m- * u-   with u+ = relu(W1)@W2, u- = relu(-W1)@W2
    h2 = relu(p[...,None]*u+ + q[...,None]*u-),  p = A_hat@m+, q = A_hat@m-
  so the (B,T,L,G) tensors never need to exist.

Sharding: data-parallel over B*L = 480 LSTM sequences -> 60 per core
  (core c: batch b=c//2, lines l0=(c%2)*60 .. +60).  All weights replicated.

Layout: everything feature-on-partition ("transposed") so the LSTM
  recurrence h_t -> gates_{t+1} needs no per-step transposes:
    hT, cT:  [125, 4, 60]   (HL=500 = 4 tiles of 125 partitions)
    gatesT:  [125, 16, 60]  (4H=2000 = 16 m-tiles)
"""

import sys

sys.path.insert(0, "/opt/trn_rl_repo")

import numpy as np
import ml_dtypes

import concourse.bass as bass
import concourse.mybir as mybir
import concourse.tile as tile
from concourse import bacc
from concourse.bass_utils import run_bass_kernel_spmd
from concourse.masks import make_identity

BF16 = ml_dtypes.bfloat16
F32 = mybir.dt.float32
BF = mybir.dt.bfloat16
AF = mybir.ActivationFunctionType
ALU = mybir.AluOpType

B, T, L, G, HL, OUT = 4, 192, 120, 500, 500, 24
H4 = 4 * HL  # 2000
NCORES = 8
NLOC = 60  # lines per core
CH = 8  # LSTM steps per production chunk
NCH = T // CH  # 24 chunks
KT = 4  # 500 = 4 k-tiles of 125
MT = 16  # 2000 = 16 m-tiles of 125
P = 125  # partition tile


def _build_program():
    nc = bacc.Bacc(
        "TRN2",
        target_bir_lowering=False,
        debug=False,
        enable_asserts=True,
        num_devices=NCORES,
    )

    xb = nc.declare_dram_parameter("xb", [2, 96, L], F32, isOutput=False)
    a_hat = nc.declare_dram_parameter("a_hat", [L, L], F32, isOutput=False)
    uu = nc.declare_dram_parameter("uu", [2, G], BF, isOutput=False)
    wih_t = nc.declare_dram_parameter("wih_t", [P, KT, H4], BF, isOutput=False)
    whh_t = nc.declare_dram_parameter("whh_t", [P, KT, H4], BF, isOutput=False)
    bias16 = nc.declare_dram_parameter("bias16", [P, MT], F32, isOutput=False)
    wh1 = nc.declare_dram_parameter("wh1", [P, 4, 3000], BF, isOutput=False)
    wh2 = nc.declare_dram_parameter("wh2", [P, 24, 1000], BF, isOutput=False)
    wh3 = nc.declare_dram_parameter("wh3", [P, 8, 3000], BF, isOutput=False)
    wh4 = nc.declare_dram_parameter("wh4", [P, 24, OUT], BF, isOutput=False)
    bh1s = nc.declare_dram_parameter("bh1s", [P, 24], F32, isOutput=False)
    bh2s = nc.declare_dram_parameter("bh2s", [P, 8], F32, isOutput=False)
    bh3s = nc.declare_dram_parameter("bh3s", [P, 24], F32, isOutput=False)
    bh4s = nc.declare_dram_parameter("bh4s", [OUT, 1], F32, isOutput=False)
    out = nc.declare_dram_parameter("out", [OUT, NLOC], F32, isOutput=True)

    # per-core DRAM scratch for p/q (t-major so chunks slice rows)
    p_dram = nc.dram_tensor("p_dram", [T, L], BF)
    q_dram = nc.dram_tensor("q_dram", [T, L], BF)

    with tile.TileContext(nc) as tc:
        with (
            tc.tile_pool(name="const", bufs=1) as constp,
            tc.tile_pool(name="state", bufs=1) as statep,
            tc.tile_pool(name="headw", bufs=1) as headwp,
        ):
            # ---- constants ----
            a_sb = constp.tile([L, L], F32)
            nc.sync.dma_start(out=a_sb, in_=a_hat[:, :])
            uu_sb = constp.tile([2, G], BF)
            nc.sync.dma_start(out=uu_sb, in_=uu[:, :])
            wih_sb = constp.tile([P, KT, H4], BF)
            nc.sync.dma_start(out=wih_sb, in_=wih_t[:, :, :])
            whh_sb = constp.tile([P, KT, H4], BF)
            nc.sync.dma_start(out=whh_sb, in_=whh_t[:, :, :])
            b16_sb = constp.tile([P, MT], F32)
            nc.sync.dma_start(out=b16_sb, in_=bias16[:, :])
            ident = constp.tile([128, 128], F32)
            make_identity(nc, ident)

            # head weights (wh1/wh4 resident; wh2/wh3 streamed in the head
            # phase — too big to keep alongside the LSTM working set)
            wh1_sb = headwp.tile([P, 4, 3000], BF)
            nc.sync.dma_start(out=wh1_sb, in_=wh1[:, :, :])
            wh4_sb = headwp.tile([P, 24, OUT], BF)
            nc.sync.dma_start(out=wh4_sb, in_=wh4[:, :, :])
            bh1_sb = headwp.tile([P, 24], F32)
            nc.sync.dma_start(out=bh1_sb, in_=bh1s[:, :])
            bh2_sb = headwp.tile([P, 8], F32)
            nc.sync.dma_start(out=bh2_sb, in_=bh2s[:, :])
            bh3_sb = headwp.tile([P, 24], F32)
            nc.sync.dma_start(out=bh3_sb, in_=bh3s[:, :])
            bh4_sb = headwp.tile([OUT, 1], F32)
            nc.sync.dma_start(out=bh4_sb, in_=bh4s[:, :])

            # ---- LSTM state ----
            hT = statep.tile([P, KT, NLOC], BF)
            cT = statep.tile([P, KT, NLOC], F32)
            nc.vector.memset(hT, 0.0)
            nc.vector.memset(cT, 0.0)

            # ================= GCN (tiny) =================
            with (
                tc.tile_pool(name="gcn", bufs=2) as gcnp,
                tc.tile_pool(name="gcn1", bufs=1) as gcn1p,
                tc.tile_pool(name="gcn_ps", bufs=2, space="PSUM") as gcnps,
            ):
                xT_sb = gcn1p.tile([L, T], F32)
                for i in range(2):
                    xt = gcnp.tile([96, L], F32, tag="xt")
                    nc.sync.dma_start(out=xt, in_=xb[i])
                    xT_ps = gcnps.tile([L, 96], F32, tag="tp")
                    nc.tensor.transpose(xT_ps, xt, ident[:96, :96])
                    nc.scalar.copy(xT_sb[:, i * 96 : (i + 1) * 96], xT_ps)
                mT_ps = gcnps.tile([L, T], F32, tag="mm")
                nc.tensor.matmul(mT_ps, lhsT=a_sb, rhs=xT_sb, start=True, stop=True)
                mp_sb = gcn1p.tile([L, T], F32)
                mm_sb = gcn1p.tile([L, T], F32)
                nc.scalar.activation(mp_sb, mT_ps, AF.Relu)
                nc.scalar.activation(mm_sb, mT_ps, AF.Relu, scale=-1.0)
                for src, dst in ((mp_sb, p_dram), (mm_sb, q_dram)):
                    rT_ps = gcnps.tile([L, T], F32, tag="mm")
                    nc.tensor.matmul(rT_ps, lhsT=a_sb, rhs=src, start=True, stop=True)
                    rT_sb = gcnp.tile([L, T], F32, tag="rt")
                    nc.scalar.copy(rT_sb, rT_ps)
                    for i in range(2):
                        r_ps = gcnps.tile([96, L], F32, tag="tp2")
                        nc.tensor.transpose(
                            r_ps, rT_sb[:, i * 96 : (i + 1) * 96], ident[:L, :L]
                        )
                        r_sb = gcnp.tile([96, L], BF, tag="rsb")
                        nc.scalar.copy(r_sb, r_ps)
                        nc.sync.dma_start(
                            out=dst[i * 96 : (i + 1) * 96, :], in_=r_sb
                        )

            # ============ production + LSTM ============
            with (
                tc.tile_pool(name="pq", bufs=2) as pqp,
                tc.tile_pool(name="h2", bufs=2) as h2p,
                tc.tile_pool(name="gx", bufs=2) as gxp,
                tc.tile_pool(name="ltmp", bufs=3) as ltp,
                tc.tile_pool(name="h2_ps", bufs=2, space="PSUM") as h2ps,
                tc.tile_pool(name="gx_ps", bufs=2, space="PSUM") as gxps,
                tc.tile_pool(name="rec_ps", bufs=2, space="PSUM") as recps,
            ):
                for c in range(NCH):
                    # p/q rows for this chunk: (2, CH, NLOC) bf16.  The host
                    # permutes lines so this core's 60 lines are 0..59.
                    pq = pqp.tile([2, CH, NLOC], BF)
                    nc.sync.dma_start(
                        out=pq[0:1],
                        in_=p_dram[c * CH : (c + 1) * CH, 0:NLOC][None],
                    )
                    nc.sync.dma_start(
                        out=pq[1:2],
                        in_=q_dram[c * CH : (c + 1) * CH, 0:NLOC][None],
                    )
                    # h2T = relu(u+ p + u- q): 4 g-tiles
                    h2 = h2p.tile([P, KT, CH * NLOC], BF)
                    for gt in range(KT):
                        h2_ps = h2ps.tile([P, CH * NLOC], F32, tag="h2ps")
                        nc.tensor.matmul(
                            h2_ps,
                            lhsT=uu_sb[:, gt * P : (gt + 1) * P],
                            rhs=pq,
                            start=True,
                            stop=True,
                        )
                        nc.scalar.activation(h2[:, gt], h2_ps, AF.Relu)
                    # gxT = W_ih @ h2 + bias
                    gx = gxp.tile([P, MT, CH * NLOC], BF)
                    for m in range(MT):
                        g_ps = gxps.tile([P, CH * NLOC], F32, tag="gps")
                        for k in range(KT):
                            nc.tensor.matmul(
                                g_ps,
                                lhsT=wih_sb[:, k, m * P : (m + 1) * P],
                                rhs=h2[:, k],
                                start=(k == 0),
                                stop=(k == KT - 1),
                            )
                        nc.vector.tensor_scalar(
                            out=gx[:, m],
                            in0=g_ps,
                            scalar1=b16_sb[:, m : m + 1],
                            scalar2=None,
                            op0=ALU.add,
                        )
                    # ---- LSTM steps ----
                    for s in range(CH):
                        rp = recps.tile([P, MT, 64], F32, tag="rec")
                        for m in range(MT):
                            for k in range(KT):
                                nc.tensor.matmul(
                                    rp[:, m, 0:NLOC],
                                    lhsT=whh_sb[:, k, m * P : (m + 1) * P],
                                    rhs=hT[:, k],
                                    start=(k == 0),
                                    stop=(k == KT - 1),
                                )
                        gtot = ltp.tile([P, MT, NLOC], F32, tag="gtot")
                        nc.vector.tensor_tensor(
                            gtot,
                            rp[:, :, 0:NLOC],
                            gx[:, :, s * NLOC : (s + 1) * NLOC],
                            op=ALU.add,
                        )
                        sif = ltp.tile([P, 8, NLOC], F32, tag="sif")
                        tg = ltp.tile([P, 4, NLOC], F32, tag="tg")
                        so = ltp.tile([P, 4, NLOC], F32, tag="so")
                        tc_ = ltp.tile([P, 4, NLOC], F32, tag="tc")
                        t1 = ltp.tile([P, 4, NLOC], F32, tag="t1")
                        t2 = ltp.tile([P, 4, NLOC], F32, tag="t2")
                        nc.scalar.activation(sif, gtot[:, 0:8], AF.Sigmoid)
                        nc.scalar.activation(tg, gtot[:, 8:12], AF.Tanh)
                        nc.scalar.activation(so, gtot[:, 12:16], AF.Sigmoid)
                        nc.vector.tensor_tensor(t1, sif[:, 4:8], cT, op=ALU.mult)
                        nc.vector.tensor_tensor(t2, sif[:, 0:4], tg, op=ALU.mult)
                        nc.vector.tensor_add(cT, t1, t2)
                        nc.scalar.activation(tc_, cT, AF.Tanh)
                        nc.vector.tensor_tensor(hT, so, tc_, op=ALU.mult)

            # ================= head =================
            with (
                tc.tile_pool(name="hd", bufs=2) as hdp,
                tc.tile_pool(name="hd1", bufs=1) as hd1p,
                tc.tile_pool(name="hd_ps", bufs=4, space="PSUM") as hdps,
            ):
                z1 = hd1p.tile([P, 24, NLOC], BF)
                for m in range(24):
                    ps = hdps.tile([P, NLOC], F32, tag="zps")
                    for k in range(4):
                        nc.tensor.matmul(
                            ps,
                            lhsT=wh1_sb[:, k, m * P : (m + 1) * P],
                            rhs=hT[:, k],
                            start=(k == 0),
                            stop=(k == 3),
                        )
                    nc.scalar.activation(
                        z1[:, m], ps, AF.Relu, bias=bh1_sb[:, m : m + 1]
                    )
                z2 = hd1p.tile([P, 8, NLOC], BF)
                for m in range(8):
                    w2t = hdp.tile([P, 24, P], BF, tag="w2t")
                    nc.sync.dma_start(out=w2t, in_=wh2[:, :, m * P : (m + 1) * P])
                    ps = hdps.tile([P, NLOC], F32, tag="zps")
                    for k in range(24):
                        nc.tensor.matmul(
                            ps,
                            lhsT=w2t[:, k],
                            rhs=z1[:, k],
                            start=(k == 0),
                            stop=(k == 23),
                        )
                    nc.scalar.activation(
                        z2[:, m], ps, AF.Relu, bias=bh2_sb[:, m : m + 1]
                    )
                z3 = hd1p.tile([P, 24, NLOC], BF)
                for m in range(24):
                    w3t = hdp.tile([P, 8, P], BF, tag="w3t")
                    nc.sync.dma_start(out=w3t, in_=wh3[:, :, m * P : (m + 1) * P])
                    ps = hdps.tile([P, NLOC], F32, tag="zps")
                    for k in range(8):
                        nc.tensor.matmul(
                            ps,
                            lhsT=w3t[:, k],
                            rhs=z2[:, k],
                            start=(k == 0),
                            stop=(k == 7),
                        )
                    nc.scalar.activation(
                        z3[:, m], ps, AF.Relu, bias=bh3_sb[:, m : m + 1]
                    )
                ps4 = hdps.tile([OUT, NLOC], F32, tag="z4")
                for k in range(24):
                    nc.tensor.matmul(
                        ps4,
                        lhsT=wh4_sb[:, k],
                        rhs=z3[:, k],
                        start=(k == 0),
                        stop=(k == 23),
                    )
                y_sb = hd1p.tile([OUT, NLOC], F32)
                nc.scalar.activation(y_sb, ps4, AF.Sigmoid, bias=bh4_sb[:, 0:1])
                nc.sync.dma_start(out=out[:, :], in_=y_sb)

    nc.compile()
    return nc


_PROG = None
_LAST_RESULTS = None


def _get_program():
    global _PROG
    if _PROG is None:
        _PROG = _build_program()
    return _PROG


def _prep(W1, W2, W_ih, W_hh, b_ih, b_hh, Wh1, Wh2, Wh3, Wh4):
    f = np.float32
    u_plus = np.maximum(W1[0].astype(f), 0) @ W2.astype(f)  # (G,)
    u_minus = np.maximum(-W1[0].astype(f), 0) @ W2.astype(f)
    uu = np.stack([u_plus, u_minus]).astype(BF16)  # (2, G)

    def kstack(wT, kt):  # (K, M) -> (P, kt, M), K = kt*P
        K, M = wT.shape
        return np.ascontiguousarray(
            wT.reshape(kt, P, M).transpose(1, 0, 2)
        )

    wih_t = kstack(np.ascontiguousarray(W_ih.T), KT).astype(BF16)  # (125,4,2000)
    whh_t = kstack(np.ascontiguousarray(W_hh.T), KT).astype(BF16)
    bias = (b_ih + b_hh).astype(f)
    bias16 = np.ascontiguousarray(bias.reshape(MT, P).T)  # (125,16)
    wh1 = kstack(Wh1.astype(f), 4).astype(BF16)
    wh2 = kstack(Wh2.astype(f), 24).astype(BF16)
    wh3 = kstack(Wh3.astype(f), 8).astype(BF16)
    wh4 = kstack(Wh4.astype(f), 24).astype(BF16)
    return uu, wih_t, whh_t, bias16, wh1, wh2, wh3, wh4


def kernel(
    x,
    A_hat,
    W1,
    W2,
    W_ih,
    W_hh,
    b_ih,
    b_hh,
    Wh1,
    bh1,
    Wh2,
    bh2,
    Wh3,
    bh3,
    Wh4,
    bh4,
):
    f = np.float32
    x = np.asarray(x, f)
    nc = _get_program()
    uu, wih_t, whh_t, bias16, wh1, wh2, wh3, wh4 = _prep(
        np.asarray(W1, f),
        np.asarray(W2, f),
        np.asarray(W_ih, f),
        np.asarray(W_hh, f),
        np.asarray(b_ih, f),
        np.asarray(b_hh, f),
        np.asarray(Wh1, f),
        np.asarray(Wh2, f),
        np.asarray(Wh3, f),
        np.asarray(Wh4, f),
    )
    a_hat = np.ascontiguousarray(np.asarray(A_hat, f))
    bh1s = np.ascontiguousarray(np.asarray(bh1, f).reshape(24, P).T)
    bh2s = np.ascontiguousarray(np.asarray(bh2, f).reshape(8, P).T)
    bh3s = np.ascontiguousarray(np.asarray(bh3, f).reshape(24, P).T)
    bh4s = np.ascontiguousarray(np.asarray(bh4, f).reshape(OUT, 1))

    # odd cores handle lines 60..119: roll lines so theirs sit at 0..59
    # (the GCN is permutation-equivariant when A_hat is permuted to match)
    a_roll = np.ascontiguousarray(np.roll(np.roll(a_hat, -NLOC, 0), -NLOC, 1))
    in_maps = []
    for c in range(NCORES):
        b = c // 2
        if c % 2 == 0:
            xc, ac = x[b], a_hat
        else:
            xc, ac = np.roll(x[b], -NLOC, axis=-1), a_roll
        in_maps.append(
            {
                "xb": np.ascontiguousarray(xc.reshape(2, 96, L)),
                "a_hat": ac,
                "uu": uu,
                "wih_t": wih_t,
                "whh_t": whh_t,
                "bias16": bias16,
                "wh1": wh1,
                "wh2": wh2,
                "wh3": wh3,
                "wh4": wh4,
                "bh1s": bh1s,
                "bh2s": bh2s,
                "bh3s": bh3s,
                "bh4s": bh4s,
            }
        )

    global _LAST_RESULTS
    _LAST_RESULTS = run_bass_kernel_spmd(nc, in_maps, list(range(NCORES)))
    res = _LAST_RESULTS.results
    y = np.zeros((B, OUT, L), f)
    for c in range(NCORES):
        b = c // 2
        l0 = (c % 2) * NLOC
        y[b, :, l0 : l0 + NLOC] = res[c]["out"]
    return y


# revision 15
# speedup vs baseline: 1.4917x; 1.4917x over previous
"""GCN-LSTM regressor as a Bass/Tile kernel for 8 Trainium2 NeuronCores.

Math restructuring (exact, up to fp reassociation):
  The reference GCN is rank-2 in disguise:
    m  = A_hat @ x_bt          (over lines)         (B,T,L)
    h1 = relu(m[...,None] * W1) ;  xw2 = h1 @ W2
       = m+ * u+  +  m- * u-   with u+ = relu(W1)@W2, u- = relu(-W1)@W2
    h2 = relu(p[...,None]*u+ + q[...,None]*u-),  p = A_hat@m+, q = A_hat@m-
  so the (B,T,L,G) tensors never need to exist.

Sharding: data-parallel over B*L = 480 LSTM sequences -> 60 per core
  (core c: batch b=c//2, lines l0=(c%2)*60 .. +60).  All weights replicated.

Layout: everything feature-on-partition ("transposed") so the LSTM
  recurrence h_t -> gates_{t+1} needs no per-step transposes.  All feature
  dims are zero-padded to multiples of 128 (HL 500->512, 4H 2000->2048,
  G 500->512, head 3000->3072, 1000->1024) so every matmul runs a full
  (128,128) stationary tile with fast-weight-load; padded lanes stay
  exactly 0 through the whole network (biases pad to 0 and sigmoid(0)*0
  terms vanish).

Schedule: gates_x production for chunk c+1 is emitted inside the step
  loop of chunk c so the tensor engine never idles during the per-step
  activation tail (keeps the PE clock un-throttled).
"""

import sys

sys.path.insert(0, "/opt/trn_rl_repo")

import numpy as np
import ml_dtypes

import concourse.bass as bass
import concourse.mybir as mybir
import concourse.tile as tile
from concourse import bacc
from concourse.bass_utils import run_bass_kernel_spmd
from concourse.masks import make_identity

BF16 = ml_dtypes.bfloat16
F32 = mybir.dt.float32
BF = mybir.dt.bfloat16
AF = mybir.ActivationFunctionType
ALU = mybir.AluOpType

B, T, L, G, HL, OUT = 4, 192, 120, 500, 500, 24
NCORES = 8
NLOC = 60  # lines per core
CH = 8  # LSTM steps per production chunk
NCH = T // CH  # 24 chunks
P = 128  # tile edge
KT = 4  # 512 = 4 k-tiles of 128
MT = 16  # 2048 = 16 m-tiles of 128
HLP, H4P, GP = 512, 2048, 512
F1, F2, F3 = 3072, 1024, 3072
NPOS = CH * NLOC  # 480 positions per chunk


def _build_program():
    nc = bacc.Bacc(
        "TRN2",
        target_bir_lowering=False,
        debug=False,
        enable_asserts=True,
        num_devices=NCORES,
    )

    xb = nc.declare_dram_parameter("xb", [2, 96, L], F32, isOutput=False)
    a_hat = nc.declare_dram_parameter("a_hat", [L, L], F32, isOutput=False)
    uu = nc.declare_dram_parameter("uu", [2, GP], BF, isOutput=False)
    wih_t = nc.declare_dram_parameter("wih_t", [P, KT, H4P], BF, isOutput=False)
    whh_t = nc.declare_dram_parameter("whh_t", [P, KT, H4P], BF, isOutput=False)
    bias16 = nc.declare_dram_parameter("bias16", [P, MT], F32, isOutput=False)
    wh1 = nc.declare_dram_parameter("wh1", [P, 4, F1], BF, isOutput=False)
    wh2 = nc.declare_dram_parameter("wh2", [P, F1 // P, F2], BF, isOutput=False)
    wh3 = nc.declare_dram_parameter("wh3", [P, F2 // P, F3], BF, isOutput=False)
    wh4 = nc.declare_dram_parameter("wh4", [P, F3 // P, OUT], BF, isOutput=False)
    bh1s = nc.declare_dram_parameter("bh1s", [P, F1 // P], F32, isOutput=False)
    bh2s = nc.declare_dram_parameter("bh2s", [P, F2 // P], F32, isOutput=False)
    bh3s = nc.declare_dram_parameter("bh3s", [P, F3 // P], F32, isOutput=False)
    bh4s = nc.declare_dram_parameter("bh4s", [OUT, 1], F32, isOutput=False)
    out = nc.declare_dram_parameter("out", [OUT, NLOC], F32, isOutput=True)

    # per-core DRAM scratch for p/q (t-major so chunks slice rows)
    p_dram = nc.dram_tensor("p_dram", [T, L], BF)
    q_dram = nc.dram_tensor("q_dram", [T, L], BF)

    with tile.TileContext(nc) as tc:
        with (
            tc.tile_pool(name="const", bufs=1) as constp,
            tc.tile_pool(name="state", bufs=1) as statep,
            tc.tile_pool(name="headw", bufs=1) as headwp,
        ):
            # ---- constants ----
            a_sb = constp.tile([L, L], F32)
            nc.sync.dma_start(out=a_sb, in_=a_hat[:, :])
            uu_sb = constp.tile([2, GP], BF)
            nc.sync.dma_start(out=uu_sb, in_=uu[:, :])
            wih_sb = constp.tile([P, KT, H4P], BF)
            nc.sync.dma_start(out=wih_sb, in_=wih_t[:, :, :])
            whh_sb = constp.tile([P, KT, H4P], BF)
            nc.sync.dma_start(out=whh_sb, in_=whh_t[:, :, :])
            b16_sb = constp.tile([P, MT], F32)
            nc.sync.dma_start(out=b16_sb, in_=bias16[:, :])
            ident = constp.tile([128, 128], F32)
            make_identity(nc, ident)

            # resident head weights (wh2/wh3 streamed in the head phase)
            wh1_sb = headwp.tile([P, 4, F1], BF)
            nc.sync.dma_start(out=wh1_sb, in_=wh1[:, :, :])
            wh4_sb = headwp.tile([P, F3 // P, OUT], BF)
            nc.sync.dma_start(out=wh4_sb, in_=wh4[:, :, :])
            bh1_sb = headwp.tile([P, F1 // P], F32)
            nc.sync.dma_start(out=bh1_sb, in_=bh1s[:, :])
            bh2_sb = headwp.tile([P, F2 // P], F32)
            nc.sync.dma_start(out=bh2_sb, in_=bh2s[:, :])
            bh3_sb = headwp.tile([P, F3 // P], F32)
            nc.sync.dma_start(out=bh3_sb, in_=bh3s[:, :])
            bh4_sb = headwp.tile([OUT, 1], F32)
            nc.sync.dma_start(out=bh4_sb, in_=bh4s[:, :])

            # ---- LSTM state ----
            hT = statep.tile([P, KT, NLOC], BF)
            cT = statep.tile([P, KT, NLOC], F32)
            nc.vector.memset(hT, 0.0)
            nc.vector.memset(cT, 0.0)

            # ================= GCN (tiny) =================
            with (
                tc.tile_pool(name="gcn", bufs=2) as gcnp,
                tc.tile_pool(name="gcn1", bufs=1) as gcn1p,
                tc.tile_pool(name="gcn_ps", bufs=2, space="PSUM") as gcnps,
            ):
                xT_sb = gcn1p.tile([L, T], F32)
                for i in range(2):
                    xt = gcnp.tile([96, L], F32, tag="xt")
                    nc.sync.dma_start(out=xt, in_=xb[i])
                    xT_ps = gcnps.tile([L, 96], F32, tag="tp")
                    nc.tensor.transpose(xT_ps, xt, ident[:96, :96])
                    nc.scalar.copy(xT_sb[:, i * 96 : (i + 1) * 96], xT_ps)
                mT_ps = gcnps.tile([L, T], F32, tag="mm")
                nc.tensor.matmul(mT_ps, lhsT=a_sb, rhs=xT_sb, start=True, stop=True)
                mp_sb = gcn1p.tile([L, T], F32)
                mm_sb = gcn1p.tile([L, T], F32)
                nc.scalar.activation(mp_sb, mT_ps, AF.Relu)
                nc.scalar.activation(mm_sb, mT_ps, AF.Relu, scale=-1.0)
                for src, dst in ((mp_sb, p_dram), (mm_sb, q_dram)):
                    rT_ps = gcnps.tile([L, T], F32, tag="mm")
                    nc.tensor.matmul(rT_ps, lhsT=a_sb, rhs=src, start=True, stop=True)
                    rT_sb = gcnp.tile([L, T], F32, tag="rt")
                    nc.scalar.copy(rT_sb, rT_ps)
                    for i in range(2):
                        r_ps = gcnps.tile([96, L], F32, tag="tp2")
                        nc.tensor.transpose(
                            r_ps, rT_sb[:, i * 96 : (i + 1) * 96], ident[:L, :L]
                        )
                        r_sb = gcnp.tile([96, L], BF, tag="rsb")
                        nc.scalar.copy(r_sb, r_ps)
                        nc.sync.dma_start(out=dst[i * 96 : (i + 1) * 96, :], in_=r_sb)

            # ============ production + LSTM ============
            with (
                tc.tile_pool(name="pq", bufs=2) as pqp,
                tc.tile_pool(name="h2", bufs=2) as h2p,
                tc.tile_pool(name="gx", bufs=2) as gxp,
                tc.tile_pool(name="ltmp", bufs=3) as ltp,
                tc.tile_pool(name="h2_ps", bufs=2, space="PSUM") as h2ps,
                tc.tile_pool(name="gx_ps", bufs=2, space="PSUM") as gxps,
                tc.tile_pool(name="rec_ps", bufs=2, space="PSUM") as recps,
            ):
                h2_tiles = [None] * NCH
                gx_tiles = [None] * NCH

                def produce_h2(c):
                    """pq DMA + h2T = relu(u+ p + u- q) for chunk c."""
                    pq = pqp.tile([2, CH, NLOC], BF, tag="pq")
                    nc.sync.dma_start(
                        out=pq[0:1], in_=p_dram[c * CH : (c + 1) * CH, 0:NLOC][None]
                    )
                    nc.sync.dma_start(
                        out=pq[1:2], in_=q_dram[c * CH : (c + 1) * CH, 0:NLOC][None]
                    )
                    h2 = h2p.tile([P, KT, NPOS], BF, tag="h2")
                    for gt in range(KT):
                        h2_ps = h2ps.tile([P, NPOS], F32, tag="h2ps")
                        nc.tensor.matmul(
                            h2_ps,
                            lhsT=uu_sb[:, gt * P : (gt + 1) * P],
                            rhs=pq,
                            start=True,
                            stop=True,
                        )
                        nc.scalar.activation(h2[:, gt], h2_ps, AF.Relu)
                    h2_tiles[c] = h2

                def produce_gx_mtile(c, m):
                    """one m-tile of gxT = W_ih @ h2 + bias for chunk c."""
                    if m == 0:
                        gx_tiles[c] = gxp.tile([P, MT, NPOS], BF, tag="gx", name="gx")
                    gx = gx_tiles[c]
                    h2 = h2_tiles[c]
                    g_ps = gxps.tile([P, NPOS], F32, tag="gps")
                    for k in range(KT):
                        nc.tensor.matmul(
                            g_ps,
                            lhsT=wih_sb[:, k, m * P : (m + 1) * P],
                            rhs=h2[:, k],
                            start=(k == 0),
                            stop=(k == KT - 1),
                        )
                    nc.vector.tensor_scalar(
                        out=gx[:, m],
                        in0=g_ps,
                        scalar1=b16_sb[:, m : m + 1],
                        scalar2=None,
                        op0=ALU.add,
                    )

                # prologue: chunk 0 production runs un-overlapped
                produce_h2(0)
                for m in range(MT):
                    produce_gx_mtile(0, m)

                for c in range(NCH):
                    gx = gx_tiles[c]
                    for s in range(CH):
                        rp = recps.tile([P, MT, 64], F32, tag="rec")
                        # --- i/f gate matmuls (psum bank 0) ---
                        for m in range(8):
                            for k in range(KT):
                                nc.tensor.matmul(
                                    rp[:, m, 0:NLOC],
                                    lhsT=whh_sb[:, k, m * P : (m + 1) * P],
                                    rhs=hT[:, k],
                                    start=(k == 0),
                                    stop=(k == KT - 1),
                                )
                        # i/f activations overlap the g/o matmuls below
                        gtif = ltp.tile([P, 8, NLOC], F32, tag="gtif")
                        nc.vector.tensor_tensor(
                            gtif,
                            rp[:, 0:8, 0:NLOC],
                            gx[:, 0:8, s * NLOC : (s + 1) * NLOC],
                            op=ALU.add,
                        )
                        sif = ltp.tile([P, 8, NLOC], F32, tag="sif")
                        nc.scalar.activation(sif, gtif, AF.Sigmoid)
                        t1 = ltp.tile([P, 4, NLOC], F32, tag="t1")
                        nc.vector.tensor_tensor(t1, sif[:, 4:8], cT, op=ALU.mult)
                        # --- g/o gate matmuls (psum bank 1) ---
                        for m in range(8, MT):
                            for k in range(KT):
                                nc.tensor.matmul(
                                    rp[:, m, 0:NLOC],
                                    lhsT=whh_sb[:, k, m * P : (m + 1) * P],
                                    rhs=hT[:, k],
                                    start=(k == 0),
                                    stop=(k == KT - 1),
                                )
                        # --- PE gap-filler: produce next chunk's gates_x ---
                        if c + 1 < NCH:
                            if s == 0:
                                produce_h2(c + 1)
                            produce_gx_mtile(c + 1, 2 * s)
                            produce_gx_mtile(c + 1, 2 * s + 1)
                        # --- g/o activations + state update (serial tail) ---
                        gtgo = ltp.tile([P, 8, NLOC], F32, tag="gtgo")
                        nc.vector.tensor_tensor(
                            gtgo,
                            rp[:, 8:16, 0:NLOC],
                            gx[:, 8:16, s * NLOC : (s + 1) * NLOC],
                            op=ALU.add,
                        )
                        tg = ltp.tile([P, 4, NLOC], F32, tag="tg")
                        nc.scalar.activation(tg, gtgo[:, 0:4], AF.Tanh)
                        so = ltp.tile([P, 4, NLOC], F32, tag="so")
                        nc.scalar.activation(so, gtgo[:, 4:8], AF.Sigmoid)
                        t2 = ltp.tile([P, 4, NLOC], F32, tag="t2")
                        nc.vector.tensor_tensor(t2, sif[:, 0:4], tg, op=ALU.mult)
                        nc.vector.tensor_add(cT, t1, t2)
                        tc_ = ltp.tile([P, 4, NLOC], F32, tag="tc")
                        nc.scalar.activation(tc_, cT, AF.Tanh)
                        nc.vector.tensor_tensor(hT, so, tc_, op=ALU.mult)

            # ================= head =================
            with (
                tc.tile_pool(name="hd", bufs=3) as hdp,
                tc.tile_pool(name="hd1", bufs=1) as hd1p,
                tc.tile_pool(name="hd_ps", bufs=4, space="PSUM") as hdps,
            ):
                z1 = hd1p.tile([P, F1 // P, NLOC], BF)
                for m in range(F1 // P):
                    ps = hdps.tile([P, NLOC], F32, tag="zps")
                    for k in range(4):
                        nc.tensor.matmul(
                            ps,
                            lhsT=wh1_sb[:, k, m * P : (m + 1) * P],
                            rhs=hT[:, k],
                            start=(k == 0),
                            stop=(k == 3),
                        )
                    nc.scalar.activation(
                        z1[:, m], ps, AF.Relu, bias=bh1_sb[:, m : m + 1]
                    )
                z2 = hd1p.tile([P, F2 // P, NLOC], BF)
                for m in range(F2 // P):
                    w2t = hdp.tile([P, F1 // P, P], BF, tag="w2t")
                    nc.sync.dma_start(out=w2t, in_=wh2[:, :, m * P : (m + 1) * P])
                    ps = hdps.tile([P, NLOC], F32, tag="zps")
                    for k in range(F1 // P):
                        nc.tensor.matmul(
                            ps,
                            lhsT=w2t[:, k],
                            rhs=z1[:, k],
                            start=(k == 0),
                            stop=(k == F1 // P - 1),
                        )
                    nc.scalar.activation(
                        z2[:, m], ps, AF.Relu, bias=bh2_sb[:, m : m + 1]
                    )
                z3 = hd1p.tile([P, F3 // P, NLOC], BF)
                for m in range(F3 // P):
                    w3t = hdp.tile([P, F2 // P, P], BF, tag="w3t")
                    nc.sync.dma_start(out=w3t, in_=wh3[:, :, m * P : (m + 1) * P])
                    ps = hdps.tile([P, NLOC], F32, tag="zps")
                    for k in range(F2 // P):
                        nc.tensor.matmul(
                            ps,
                            lhsT=w3t[:, k],
                            rhs=z2[:, k],
                            start=(k == 0),
                            stop=(k == F2 // P - 1),
                        )
                    nc.scalar.activation(
                        z3[:, m], ps, AF.Relu, bias=bh3_sb[:, m : m + 1]
                    )
                ps4 = hdps.tile([OUT, NLOC], F32, tag="z4")
                for k in range(F3 // P):
                    nc.tensor.matmul(
                        ps4,
                        lhsT=wh4_sb[:, k],
                        rhs=z3[:, k],
                        start=(k == 0),
                        stop=(k == F3 // P - 1),
                    )
                y_sb = hd1p.tile([OUT, NLOC], F32)
                nc.scalar.activation(y_sb, ps4, AF.Sigmoid, bias=bh4_sb[:, 0:1])
                nc.sync.dma_start(out=out[:, :], in_=y_sb)

    nc.compile()
    return nc


_PROG = None
_LAST_RESULTS = None


def _get_program():
    global _PROG
    if _PROG is None:
        _PROG = _build_program()
    return _PROG


def _pad_gates(w, pad_in, pad_unit):
    """(4*HL, K) -> (4*pad_unit, pad_in) with each gate block padded."""
    H4_, K_ = w.shape
    hl = H4_ // 4
    out = np.zeros((4 * pad_unit, pad_in), w.dtype)
    for g in range(4):
        out[g * pad_unit : g * pad_unit + hl, :K_] = w[g * hl : (g + 1) * hl]
    return out


def _kstack(wT, p=P):
    """(K, M) -> (p, K//p, M) partition-major for SBUF staging."""
    K_, M_ = wT.shape
    return np.ascontiguousarray(wT.reshape(K_ // p, p, M_).transpose(1, 0, 2))


def _prep(W1, W2, W_ih, W_hh, b_ih, b_hh, Wh1, bh1, Wh2, bh2, Wh3, bh3, Wh4, bh4):
    f = np.float32
    u_plus = np.maximum(W1[0], 0) @ W2  # (G,)
    u_minus = np.maximum(-W1[0], 0) @ W2
    uu = np.zeros((2, GP), f)
    uu[0, :G] = u_plus
    uu[1, :G] = u_minus
    uu = uu.astype(BF16)

    # W_ih: (2000, 500) -> padded (2048, 512) -> T -> (512, 2048)
    wih_p = _pad_gates(W_ih, GP, HLP)  # (2048, 512)
    wih_t = _kstack(np.ascontiguousarray(wih_p.T)).astype(BF16)  # (128,4,2048)
    whh_p = _pad_gates(W_hh, HLP, HLP)  # (2048, 512)
    whh_t = _kstack(np.ascontiguousarray(whh_p.T)).astype(BF16)
    bias = np.zeros(H4P, f)
    bb = (b_ih + b_hh).astype(f)
    for g in range(4):
        bias[g * HLP : g * HLP + HL] = bb[g * HL : (g + 1) * HL]
    bias16 = np.ascontiguousarray(bias.reshape(MT, P).T)  # (128,16)

    def pad2(w, r, c):
        o = np.zeros((r, c), f)
        o[: w.shape[0], : w.shape[1]] = w
        return o

    wh1 = _kstack(pad2(Wh1, HLP, F1)).astype(BF16)
    wh2 = _kstack(pad2(Wh2, F1, F2)).astype(BF16)
    wh3 = _kstack(pad2(Wh3, F2, F3)).astype(BF16)
    wh4 = _kstack(pad2(Wh4, F3, OUT)).astype(BF16)
    bh1s = np.ascontiguousarray(pad2(bh1[None], 1, F1)[0].reshape(F1 // P, P).T)
    bh2s = np.ascontiguousarray(pad2(bh2[None], 1, F2)[0].reshape(F2 // P, P).T)
    bh3s = np.ascontiguousarray(pad2(bh3[None], 1, F3)[0].reshape(F3 // P, P).T)
    bh4s = np.ascontiguousarray(bh4.astype(f).reshape(OUT, 1))
    return uu, wih_t, whh_t, bias16, wh1, wh2, wh3, wh4, bh1s, bh2s, bh3s, bh4s


def kernel(
    x,
    A_hat,
    W1,
    W2,
    W_ih,
    W_hh,
    b_ih,
    b_hh,
    Wh1,
    bh1,
    Wh2,
    bh2,
    Wh3,
    bh3,
    Wh4,
    bh4,
):
    f = np.float32
    x = np.asarray(x, f)
    nc = _get_program()
    args = [
        np.asarray(a, f)
        for a in (
            W1,
            W2,
            W_ih,
            W_hh,
            b_ih,
            b_hh,
            Wh1,
            bh1,
            Wh2,
            bh2,
            Wh3,
            bh3,
            Wh4,
            bh4,
        )
    ]
    uu, wih_t, whh_t, bias16, wh1, wh2, wh3, wh4, bh1s, bh2s, bh3s, bh4s = _prep(*args)
    a_hat = np.ascontiguousarray(np.asarray(A_hat, f))

    # odd cores handle lines 60..119: roll lines so theirs sit at 0..59
    # (the GCN is permutation-equivariant when A_hat is permuted to match)
    a_roll = np.ascontiguousarray(np.roll(np.roll(a_hat, -NLOC, 0), -NLOC, 1))
    in_maps = []
    for c in range(NCORES):
        b = c // 2
        if c % 2 == 0:
            xc, ac = x[b], a_hat
        else:
            xc, ac = np.roll(x[b], -NLOC, axis=-1), a_roll
        in_maps.append(
            {
                "xb": np.ascontiguousarray(xc.reshape(2, 96, L)),
                "a_hat": ac,
                "uu": uu,
                "wih_t": wih_t,
                "whh_t": whh_t,
                "bias16": bias16,
                "wh1": wh1,
                "wh2": wh2,
                "wh3": wh3,
                "wh4": wh4,
                "bh1s": bh1s,
                "bh2s": bh2s,
                "bh3s": bh3s,
                "bh4s": bh4s,
            }
        )

    global _LAST_RESULTS
    _LAST_RESULTS = run_bass_kernel_spmd(nc, in_maps, list(range(NCORES)))
    res = _LAST_RESULTS.results
    y = np.zeros((B, OUT, L), f)
    for c in range(NCORES):
        b = c // 2
        l0 = (c % 2) * NLOC
        y[b, :, l0 : l0 + NLOC] = res[c]["out"]
    return y


# revision 19
# speedup vs baseline: 2.0114x; 1.3485x over previous
"""GCN-LSTM regressor as a Bass/Tile kernel for 8 Trainium2 NeuronCores.

Math restructuring (exact, up to fp reassociation):
  The reference GCN is rank-2 in disguise:
    m  = A_hat @ x_bt          (over lines)         (B,T,L)
    h1 = relu(m[...,None] * W1) ;  xw2 = h1 @ W2
       = m+ * u+  +  m- * u-   with u+ = relu(W1)@W2, u- = relu(-W1)@W2
    h2 = relu(p[...,None]*u+ + q[...,None]*u-),  p = A_hat@m+, q = A_hat@m-
  so the (B,T,L,G) tensors never need to exist.

Sharding: data-parallel over B*L = 480 LSTM sequences -> 60 per core
  (core c: batch b=c//2, lines l0=(c%2)*60 .. +60).  All weights replicated.

Layout: everything feature-on-partition ("transposed") so the LSTM
  recurrence h_t -> gates_{t+1} needs no per-step transposes.  All feature
  dims are zero-padded to multiples of 128 (HL 500->512, 4H 2000->2048,
  G 500->512, head 3000->3072, 1000->1024) so every matmul runs a full
  (128,128) stationary tile with fast-weight-load; padded lanes stay
  exactly 0 through the whole network (biases pad to 0 and sigmoid(0)*0
  terms vanish).

Schedule: gates_x production for chunk c+1 is emitted inside the step
  loop of chunk c so the tensor engine never idles during the per-step
  activation tail (keeps the PE clock un-throttled).
"""

import sys

sys.path.insert(0, "/opt/trn_rl_repo")

import numpy as np
import ml_dtypes

import concourse.bass as bass
import concourse.mybir as mybir
import concourse.tile as tile
from concourse import bacc
from concourse.bass_utils import run_bass_kernel_spmd
from concourse.masks import make_identity

BF16 = ml_dtypes.bfloat16
F32 = mybir.dt.float32
BF = mybir.dt.bfloat16
AF = mybir.ActivationFunctionType
ALU = mybir.AluOpType

B, T, L, G, HL, OUT = 4, 192, 120, 500, 500, 24
NCORES = 8
NLOC = 60  # lines per core
CH = 8  # LSTM steps per production chunk
NCH = T // CH  # 24 chunks
P = 128  # tile edge
KT = 4  # 512 = 4 k-tiles of 128
MT = 16  # 2048 = 16 m-tiles of 128
HLP, H4P, GP = 512, 2048, 512
F1, F2, F3 = 3072, 1024, 3072
NPOS = CH * NLOC  # 480 positions per chunk


def _build_program():
    nc = bacc.Bacc(
        "TRN2",
        target_bir_lowering=False,
        debug=False,
        enable_asserts=True,
        num_devices=NCORES,
    )

    xb = nc.declare_dram_parameter("xb", [2, 96, L], F32, isOutput=False)
    a_hat = nc.declare_dram_parameter("a_hat", [L, L], F32, isOutput=False)
    uu = nc.declare_dram_parameter("uu", [2, GP], BF, isOutput=False)
    wih_t = nc.declare_dram_parameter("wih_t", [P, KT, H4P], BF, isOutput=False)
    whh_t = nc.declare_dram_parameter("whh_t", [P, KT, H4P], BF, isOutput=False)
    bias16 = nc.declare_dram_parameter("bias16", [P, MT], F32, isOutput=False)
    wh1 = nc.declare_dram_parameter("wh1", [P, 4, F1], BF, isOutput=False)
    wh2 = nc.declare_dram_parameter("wh2", [P, F1 // P, F2], BF, isOutput=False)
    wh3 = nc.declare_dram_parameter("wh3", [P, F2 // P, F3], BF, isOutput=False)
    wh4 = nc.declare_dram_parameter("wh4", [P, F3 // P, OUT], BF, isOutput=False)
    bh1s = nc.declare_dram_parameter("bh1s", [P, F1 // P], F32, isOutput=False)
    bh2s = nc.declare_dram_parameter("bh2s", [P, F2 // P], F32, isOutput=False)
    bh3s = nc.declare_dram_parameter("bh3s", [P, F3 // P], F32, isOutput=False)
    bh4s = nc.declare_dram_parameter("bh4s", [OUT, 1], F32, isOutput=False)
    out = nc.declare_dram_parameter("out", [OUT, NLOC], F32, isOutput=True)

    # per-core DRAM scratch for p/q (t-major so chunks slice rows)
    p_dram = nc.dram_tensor("p_dram", [T, L], BF)
    q_dram = nc.dram_tensor("q_dram", [T, L], BF)

    with tile.TileContext(nc) as tc:
        with (
            tc.tile_pool(name="const", bufs=1) as constp,
            tc.tile_pool(name="state", bufs=1) as statep,
            tc.tile_pool(name="headw", bufs=1) as headwp,
        ):
            # ---- constants ----
            a_sb = constp.tile([L, L], F32)
            nc.sync.dma_start(out=a_sb, in_=a_hat[:, :])
            uu_sb = constp.tile([2, GP], BF)
            nc.sync.dma_start(out=uu_sb, in_=uu[:, :])
            wih_sb = constp.tile([P, KT, H4P], BF)
            nc.sync.dma_start(out=wih_sb, in_=wih_t[:, :, :])
            whh_sb = constp.tile([P, KT, H4P], BF)
            nc.sync.dma_start(out=whh_sb, in_=whh_t[:, :, :])
            b16_sb = constp.tile([P, MT], F32)
            nc.sync.dma_start(out=b16_sb, in_=bias16[:, :])
            ident = constp.tile([128, 128], F32)
            make_identity(nc, ident)

            # resident head weights (wh2/wh3 streamed in the head phase)
            wh1_sb = headwp.tile([P, 4, F1], BF)
            nc.sync.dma_start(out=wh1_sb, in_=wh1[:, :, :])
            wh4_sb = headwp.tile([P, F3 // P, OUT], BF)
            nc.sync.dma_start(out=wh4_sb, in_=wh4[:, :, :])
            bh1_sb = headwp.tile([P, F1 // P], F32)
            nc.sync.dma_start(out=bh1_sb, in_=bh1s[:, :])
            bh2_sb = headwp.tile([P, F2 // P], F32)
            nc.sync.dma_start(out=bh2_sb, in_=bh2s[:, :])
            bh3_sb = headwp.tile([P, F3 // P], F32)
            nc.sync.dma_start(out=bh3_sb, in_=bh3s[:, :])
            bh4_sb = headwp.tile([OUT, 1], F32)
            nc.sync.dma_start(out=bh4_sb, in_=bh4s[:, :])

            # ---- LSTM state ----
            hT = statep.tile([P, KT, NLOC], BF)
            cT = statep.tile([P, KT, NLOC], F32)
            nc.vector.memset(hT, 0.0)
            nc.vector.memset(cT, 0.0)

            # ================= GCN (tiny) =================
            with (
                tc.tile_pool(name="gcn", bufs=2) as gcnp,
                tc.tile_pool(name="gcn1", bufs=1) as gcn1p,
                tc.tile_pool(name="gcn_ps", bufs=2, space="PSUM") as gcnps,
            ):
                xT_sb = gcn1p.tile([L, T], F32)
                for i in range(2):
                    xt = gcnp.tile([96, L], F32, tag="xt")
                    nc.sync.dma_start(out=xt, in_=xb[i])
                    xT_ps = gcnps.tile([L, 96], F32, tag="tp")
                    nc.tensor.transpose(xT_ps, xt, ident[:96, :96])
                    nc.scalar.copy(xT_sb[:, i * 96 : (i + 1) * 96], xT_ps)
                mT_ps = gcnps.tile([L, T], F32, tag="mm")
                nc.tensor.matmul(mT_ps, lhsT=a_sb, rhs=xT_sb, start=True, stop=True)
                mp_sb = gcn1p.tile([L, T], F32)
                mm_sb = gcn1p.tile([L, T], F32)
                nc.scalar.activation(mp_sb, mT_ps, AF.Relu)
                nc.scalar.activation(mm_sb, mT_ps, AF.Relu, scale=-1.0)
                for src, dst in ((mp_sb, p_dram), (mm_sb, q_dram)):
                    rT_ps = gcnps.tile([L, T], F32, tag="mm")
                    nc.tensor.matmul(rT_ps, lhsT=a_sb, rhs=src, start=True, stop=True)
                    rT_sb = gcnp.tile([L, T], F32, tag="rt")
                    nc.scalar.copy(rT_sb, rT_ps)
                    for i in range(2):
                        r_ps = gcnps.tile([96, L], F32, tag="tp2")
                        nc.tensor.transpose(
                            r_ps, rT_sb[:, i * 96 : (i + 1) * 96], ident[:L, :L]
                        )
                        r_sb = gcnp.tile([96, L], BF, tag="rsb")
                        nc.scalar.copy(r_sb, r_ps)
                        nc.sync.dma_start(out=dst[i * 96 : (i + 1) * 96, :], in_=r_sb)

            # ============ production + LSTM ============
            with (
                tc.tile_pool(name="pq", bufs=2) as pqp,
                tc.tile_pool(name="h2", bufs=2) as h2p,
                tc.tile_pool(name="gx", bufs=2) as gxp,
                tc.tile_pool(name="ltmp", bufs=3) as ltp,
                tc.tile_pool(name="h2_ps", bufs=2, space="PSUM") as h2ps,
                tc.tile_pool(name="gx_ps", bufs=2, space="PSUM") as gxps,
                tc.tile_pool(name="rec_ps", bufs=1, space="PSUM") as recps,
            ):
                h2_tiles = [None] * NCH
                gx_tiles = [None] * NCH

                def produce_h2(c):
                    """pq DMA + h2T = relu(u+ p + u- q) for chunk c."""
                    pq = pqp.tile([2, CH, NLOC], BF, tag="pq")
                    nc.sync.dma_start(
                        out=pq[0:1], in_=p_dram[c * CH : (c + 1) * CH, 0:NLOC][None]
                    )
                    nc.sync.dma_start(
                        out=pq[1:2], in_=q_dram[c * CH : (c + 1) * CH, 0:NLOC][None]
                    )
                    h2 = h2p.tile([P, KT, NPOS], BF, tag="h2")
                    for gt in range(KT):
                        h2_ps = h2ps.tile([P, NPOS], F32, tag="h2ps")
                        nc.tensor.matmul(
                            h2_ps,
                            lhsT=uu_sb[:, gt * P : (gt + 1) * P],
                            rhs=pq,
                            start=True,
                            stop=True,
                        )
                        nc.scalar.activation(h2[:, gt], h2_ps, AF.Relu)
                    h2_tiles[c] = h2

                def produce_gx_mtile(c, m):
                    """one m-tile of gxT = W_ih @ h2 + bias for chunk c."""
                    if m == 0:
                        gx_tiles[c] = gxp.tile([P, MT, NPOS], BF, tag="gx", name="gx")
                    gx = gx_tiles[c]
                    h2 = h2_tiles[c]
                    g_ps = gxps.tile([P, NPOS], F32, tag="gps")
                    for k in range(KT):
                        nc.tensor.matmul(
                            g_ps,
                            lhsT=wih_sb[:, k, m * P : (m + 1) * P],
                            rhs=h2[:, k],
                            start=(k == 0),
                            stop=(k == KT - 1),
                        )
                    # psum->sbuf copy with bias on the scalar engine (keeps
                    # the vector engine free for the LSTM state updates)
                    nc.scalar.activation(
                        gx[:, m], g_ps, AF.Identity, bias=b16_sb[:, m : m + 1]
                    )

                # prologue: chunk 0 production runs un-overlapped
                produce_h2(0)
                for m in range(MT):
                    produce_gx_mtile(0, m)

                def rec_gate(gate, dst_ps):
                    """16 matmul pairs for one gate (4 m-tiles x 4 k)."""
                    for mi in range(4):
                        m = gate * 4 + mi
                        for k in range(KT):
                            nc.tensor.matmul(
                                dst_ps[:, mi, 0:NLOC],
                                lhsT=whh_sb[:, k, m * P : (m + 1) * P],
                                rhs=hT[:, k],
                                start=(k == 0),
                                stop=(k == KT - 1),
                            )

                def gate_total(name, dst, gate, gx, s, ps):
                    gt = ltp.tile([P, 4, NLOC], F32, tag=name, name=name)
                    nc.vector.tensor_tensor(
                        gt,
                        ps[:, 0:4, 0:NLOC],
                        gx[:, gate * 4 : gate * 4 + 4, s * NLOC : (s + 1) * NLOC],
                        op=ALU.add,
                    )
                    return gt

                # device gate order: 0=g 1=f 2=i 3=o (one PSUM bank each)
                for c in range(NCH):
                    gx = gx_tiles[c]
                    for s in range(CH):
                        rps = [
                            recps.tile(
                                [P, 4, 64], F32, tag=f"rec{gi}", name=f"rec{gi}"
                            )
                            for gi in range(4)
                        ]
                        # g gate
                        rec_gate(0, rps[0])
                        gt_g = gate_total("gtg", None, 0, gx, s, rps[0])
                        tg = ltp.tile([P, 4, NLOC], F32, tag="tg")
                        nc.scalar.activation(tg, gt_g, AF.Tanh)
                        # f gate
                        rec_gate(1, rps[1])
                        gt_f = gate_total("gtf", None, 1, gx, s, rps[1])
                        sf = ltp.tile([P, 4, NLOC], F32, tag="sf")
                        nc.scalar.activation(sf, gt_f, AF.Sigmoid)
                        t1 = ltp.tile([P, 4, NLOC], F32, tag="t1")
                        nc.vector.tensor_tensor(t1, sf, cT, op=ALU.mult)
                        # i gate
                        rec_gate(2, rps[2])
                        gt_i = gate_total("gti", None, 2, gx, s, rps[2])
                        si = ltp.tile([P, 4, NLOC], F32, tag="si")
                        nc.scalar.activation(si, gt_i, AF.Sigmoid)
                        t2 = ltp.tile([P, 4, NLOC], F32, tag="t2")
                        nc.vector.tensor_tensor(t2, si, tg, op=ALU.mult)
                        # o gate matmuls, then gap-filler, then c/h updates
                        rec_gate(3, rps[3])
                        if c + 1 < NCH:
                            if s == 0:
                                produce_h2(c + 1)
                            produce_gx_mtile(c + 1, 2 * s)
                            produce_gx_mtile(c + 1, 2 * s + 1)
                        nc.vector.tensor_add(cT, t1, t2)
                        tc_ = ltp.tile([P, 4, NLOC], F32, tag="tc")
                        nc.scalar.activation(tc_, cT, AF.Tanh)
                        gt_o = gate_total("gto", None, 3, gx, s, rps[3])
                        so = ltp.tile([P, 4, NLOC], F32, tag="so")
                        nc.scalar.activation(so, gt_o, AF.Sigmoid)
                        nc.vector.tensor_tensor(hT, so, tc_, op=ALU.mult)

            # ================= head =================
            with (
                tc.tile_pool(name="hd", bufs=3) as hdp,
                tc.tile_pool(name="hd1", bufs=1) as hd1p,
                tc.tile_pool(name="hd_ps", bufs=4, space="PSUM") as hdps,
            ):
                z1 = hd1p.tile([P, F1 // P, NLOC], BF)
                for m in range(F1 // P):
                    ps = hdps.tile([P, NLOC], F32, tag="zps")
                    for k in range(4):
                        nc.tensor.matmul(
                            ps,
                            lhsT=wh1_sb[:, k, m * P : (m + 1) * P],
                            rhs=hT[:, k],
                            start=(k == 0),
                            stop=(k == 3),
                        )
                    nc.scalar.activation(
                        z1[:, m], ps, AF.Relu, bias=bh1_sb[:, m : m + 1]
                    )
                z2 = hd1p.tile([P, F2 // P, NLOC], BF)
                for m in range(F2 // P):
                    w2t = hdp.tile([P, F1 // P, P], BF, tag="w2t")
                    nc.sync.dma_start(out=w2t, in_=wh2[:, :, m * P : (m + 1) * P])
                    ps = hdps.tile([P, NLOC], F32, tag="zps")
                    for k in range(F1 // P):
                        nc.tensor.matmul(
                            ps,
                            lhsT=w2t[:, k],
                            rhs=z1[:, k],
                            start=(k == 0),
                            stop=(k == F1 // P - 1),
                        )
                    nc.scalar.activation(
                        z2[:, m], ps, AF.Relu, bias=bh2_sb[:, m : m + 1]
                    )
                z3 = hd1p.tile([P, F3 // P, NLOC], BF)
                for m in range(F3 // P):
                    w3t = hdp.tile([P, F2 // P, P], BF, tag="w3t")
                    nc.sync.dma_start(out=w3t, in_=wh3[:, :, m * P : (m + 1) * P])
                    ps = hdps.tile([P, NLOC], F32, tag="zps")
                    for k in range(F2 // P):
                        nc.tensor.matmul(
                            ps,
                            lhsT=w3t[:, k],
                            rhs=z2[:, k],
                            start=(k == 0),
                            stop=(k == F2 // P - 1),
                        )
                    nc.scalar.activation(
                        z3[:, m], ps, AF.Relu, bias=bh3_sb[:, m : m + 1]
                    )
                ps4 = hdps.tile([OUT, NLOC], F32, tag="z4")
                for k in range(F3 // P):
                    nc.tensor.matmul(
                        ps4,
                        lhsT=wh4_sb[:, k],
                        rhs=z3[:, k],
                        start=(k == 0),
                        stop=(k == F3 // P - 1),
                    )
                y_sb = hd1p.tile([OUT, NLOC], F32)
                nc.scalar.activation(y_sb, ps4, AF.Sigmoid, bias=bh4_sb[:, 0:1])
                nc.sync.dma_start(out=out[:, :], in_=y_sb)

    nc.compile()
    return nc


_PROG = None
_LAST_RESULTS = None


def _get_program():
    global _PROG
    if _PROG is None:
        _PROG = _build_program()
    return _PROG


GATE_PERM = (2, 1, 0, 3)  # device gate order [g, f, i, o] from pytorch [i, f, g, o]


def _pad_gates(w, pad_in, pad_unit):
    """(4*HL, K) -> (4*pad_unit, pad_in), gate blocks permuted to GATE_PERM."""
    H4_, K_ = w.shape
    hl = H4_ // 4
    out = np.zeros((4 * pad_unit, pad_in), w.dtype)
    for g in range(4):
        src = GATE_PERM[g]
        out[g * pad_unit : g * pad_unit + hl, :K_] = w[src * hl : (src + 1) * hl]
    return out


def _kstack(wT, p=P):
    """(K, M) -> (p, K//p, M) partition-major for SBUF staging."""
    K_, M_ = wT.shape
    return np.ascontiguousarray(wT.reshape(K_ // p, p, M_).transpose(1, 0, 2))


def _prep(W1, W2, W_ih, W_hh, b_ih, b_hh, Wh1, bh1, Wh2, bh2, Wh3, bh3, Wh4, bh4):
    f = np.float32
    u_plus = np.maximum(W1[0], 0) @ W2  # (G,)
    u_minus = np.maximum(-W1[0], 0) @ W2
    uu = np.zeros((2, GP), f)
    uu[0, :G] = u_plus
    uu[1, :G] = u_minus
    uu = uu.astype(BF16)

    # W_ih: (2000, 500) -> padded (2048, 512) -> T -> (512, 2048)
    wih_p = _pad_gates(W_ih, GP, HLP)  # (2048, 512)
    wih_t = _kstack(np.ascontiguousarray(wih_p.T)).astype(BF16)  # (128,4,2048)
    whh_p = _pad_gates(W_hh, HLP, HLP)  # (2048, 512)
    whh_t = _kstack(np.ascontiguousarray(whh_p.T)).astype(BF16)
    bias = np.zeros(H4P, f)
    bb = (b_ih + b_hh).astype(f)
    for g in range(4):
        src = GATE_PERM[g]
        bias[g * HLP : g * HLP + HL] = bb[src * HL : (src + 1) * HL]
    bias16 = np.ascontiguousarray(bias.reshape(MT, P).T)  # (128,16)

    def pad2(w, r, c):
        o = np.zeros((r, c), f)
        o[: w.shape[0], : w.shape[1]] = w
        return o

    wh1 = _kstack(pad2(Wh1, HLP, F1)).astype(BF16)
    wh2 = _kstack(pad2(Wh2, F1, F2)).astype(BF16)
    wh3 = _kstack(pad2(Wh3, F2, F3)).astype(BF16)
    wh4 = _kstack(pad2(Wh4, F3, OUT)).astype(BF16)
    bh1s = np.ascontiguousarray(pad2(bh1[None], 1, F1)[0].reshape(F1 // P, P).T)
    bh2s = np.ascontiguousarray(pad2(bh2[None], 1, F2)[0].reshape(F2 // P, P).T)
    bh3s = np.ascontiguousarray(pad2(bh3[None], 1, F3)[0].reshape(F3 // P, P).T)
    bh4s = np.ascontiguousarray(bh4.astype(f).reshape(OUT, 1))
    return uu, wih_t, whh_t, bias16, wh1, wh2, wh3, wh4, bh1s, bh2s, bh3s, bh4s


def kernel(
    x,
    A_hat,
    W1,
    W2,
    W_ih,
    W_hh,
    b_ih,
    b_hh,
    Wh1,
    bh1,
    Wh2,
    bh2,
    Wh3,
    bh3,
    Wh4,
    bh4,
):
    f = np.float32
    x = np.asarray(x, f)
    nc = _get_program()
    args = [
        np.asarray(a, f)
        for a in (
            W1,
            W2,
            W_ih,
            W_hh,
            b_ih,
            b_hh,
            Wh1,
            bh1,
            Wh2,
            bh2,
            Wh3,
            bh3,
            Wh4,
            bh4,
        )
    ]
    uu, wih_t, whh_t, bias16, wh1, wh2, wh3, wh4, bh1s, bh2s, bh3s, bh4s = _prep(*args)
    a_hat = np.ascontiguousarray(np.asarray(A_hat, f))

    # odd cores handle lines 60..119: roll lines so theirs sit at 0..59
    # (the GCN is permutation-equivariant when A_hat is permuted to match)
    a_roll = np.ascontiguousarray(np.roll(np.roll(a_hat, -NLOC, 0), -NLOC, 1))
    in_maps = []
    for c in range(NCORES):
        b = c // 2
        if c % 2 == 0:
            xc, ac = x[b], a_hat
        else:
            xc, ac = np.roll(x[b], -NLOC, axis=-1), a_roll
        in_maps.append(
            {
                "xb": np.ascontiguousarray(xc.reshape(2, 96, L)),
                "a_hat": ac,
                "uu": uu,
                "wih_t": wih_t,
                "whh_t": whh_t,
                "bias16": bias16,
                "wh1": wh1,
                "wh2": wh2,
                "wh3": wh3,
                "wh4": wh4,
                "bh1s": bh1s,
                "bh2s": bh2s,
                "bh3s": bh3s,
                "bh4s": bh4s,
            }
        )

    global _LAST_RESULTS
    _LAST_RESULTS = run_bass_kernel_spmd(nc, in_maps, list(range(NCORES)))
    res = _LAST_RESULTS.results
    y = np.zeros((B, OUT, L), f)
    for c in range(NCORES):
        b = c // 2
        l0 = (c % 2) * NLOC
        y[b, :, l0 : l0 + NLOC] = res[c]["out"]
    return y


# revision 20
# speedup vs baseline: 2.0254x; 1.0069x over previous
"""GCN-LSTM regressor as a Bass/Tile kernel for 8 Trainium2 NeuronCores.

Math restructuring (exact, up to fp reassociation):
  The reference GCN is rank-2 in disguise:
    m  = A_hat @ x_bt          (over lines)         (B,T,L)
    h1 = relu(m[...,None] * W1) ;  xw2 = h1 @ W2
       = m+ * u+  +  m- * u-   with u+ = relu(W1)@W2, u- = relu(-W1)@W2
    h2 = relu(p[...,None]*u+ + q[...,None]*u-),  p = A_hat@m+, q = A_hat@m-
  so the (B,T,L,G) tensors never need to exist.

Sharding: data-parallel over B*L = 480 LSTM sequences -> 60 per core
  (core c: batch b=c//2, lines l0=(c%2)*60 .. +60).  All weights replicated.

Layout: everything feature-on-partition ("transposed") so the LSTM
  recurrence h_t -> gates_{t+1} needs no per-step transposes.  All feature
  dims are zero-padded to multiples of 128 (HL 500->512, 4H 2000->2048,
  G 500->512, head 3000->3072, 1000->1024) so every matmul runs a full
  (128,128) stationary tile with fast-weight-load; padded lanes stay
  exactly 0 through the whole network (biases pad to 0 and sigmoid(0)*0
  terms vanish).

Schedule: gates_x production for chunk c+1 is emitted inside the step
  loop of chunk c so the tensor engine never idles during the per-step
  activation tail (keeps the PE clock un-throttled).
"""

import sys

sys.path.insert(0, "/opt/trn_rl_repo")

import numpy as np
import ml_dtypes

import concourse.bass as bass
import concourse.mybir as mybir
import concourse.tile as tile
from concourse import bacc
from concourse.bass_utils import run_bass_kernel_spmd
from concourse.masks import make_identity

BF16 = ml_dtypes.bfloat16
FP8 = ml_dtypes.float8_e4m3
F32 = mybir.dt.float32
BF = mybir.dt.bfloat16
F8 = mybir.dt.float8e4
WHH_SCALE = 512.0
AF = mybir.ActivationFunctionType
ALU = mybir.AluOpType

B, T, L, G, HL, OUT = 4, 192, 120, 500, 500, 24
NCORES = 8
NLOC = 60  # lines per core
CH = 8  # LSTM steps per production chunk
NCH = T // CH  # 24 chunks
P = 128  # tile edge
KT = 4  # 512 = 4 k-tiles of 128
MT = 16  # 2048 = 16 m-tiles of 128
HLP, H4P, GP = 512, 2048, 512
F1, F2, F3 = 3072, 1024, 3072
NPOS = CH * NLOC  # 480 positions per chunk


def _build_program():
    nc = bacc.Bacc(
        "TRN2",
        target_bir_lowering=False,
        debug=False,
        enable_asserts=True,
        num_devices=NCORES,
    )

    xb = nc.declare_dram_parameter("xb", [2, 96, L], F32, isOutput=False)
    a_hat = nc.declare_dram_parameter("a_hat", [L, L], F32, isOutput=False)
    uu = nc.declare_dram_parameter("uu", [2, GP], BF, isOutput=False)
    wih_t = nc.declare_dram_parameter("wih_t", [P, KT, H4P], BF, isOutput=False)
    whh_t = nc.declare_dram_parameter("whh_t", [P, KT, H4P], F8, isOutput=False)
    bias16 = nc.declare_dram_parameter("bias16", [P, MT], F32, isOutput=False)
    wh1 = nc.declare_dram_parameter("wh1", [P, 4, F1], BF, isOutput=False)
    wh2 = nc.declare_dram_parameter("wh2", [P, F1 // P, F2], BF, isOutput=False)
    wh3 = nc.declare_dram_parameter("wh3", [P, F2 // P, F3], BF, isOutput=False)
    wh4 = nc.declare_dram_parameter("wh4", [P, F3 // P, OUT], BF, isOutput=False)
    bh1s = nc.declare_dram_parameter("bh1s", [P, F1 // P], F32, isOutput=False)
    bh2s = nc.declare_dram_parameter("bh2s", [P, F2 // P], F32, isOutput=False)
    bh3s = nc.declare_dram_parameter("bh3s", [P, F3 // P], F32, isOutput=False)
    bh4s = nc.declare_dram_parameter("bh4s", [OUT, 1], F32, isOutput=False)
    out = nc.declare_dram_parameter("out", [OUT, NLOC], F32, isOutput=True)

    # per-core DRAM scratch for p/q (t-major so chunks slice rows)
    p_dram = nc.dram_tensor("p_dram", [T, L], BF)
    q_dram = nc.dram_tensor("q_dram", [T, L], BF)

    with tile.TileContext(nc) as tc:
        with (
            tc.tile_pool(name="const", bufs=1) as constp,
            tc.tile_pool(name="state", bufs=1) as statep,
            tc.tile_pool(name="headw", bufs=1) as headwp,
        ):
            # ---- constants ----
            a_sb = constp.tile([L, L], F32)
            nc.sync.dma_start(out=a_sb, in_=a_hat[:, :])
            uu_sb = constp.tile([2, GP], BF)
            nc.sync.dma_start(out=uu_sb, in_=uu[:, :])
            wih_sb = constp.tile([P, KT, H4P], BF)
            nc.sync.dma_start(out=wih_sb, in_=wih_t[:, :, :])
            whh_sb = constp.tile([P, KT, H4P], F8)
            nc.sync.dma_start(out=whh_sb, in_=whh_t[:, :, :])
            b16_sb = constp.tile([P, MT], F32)
            nc.sync.dma_start(out=b16_sb, in_=bias16[:, :])
            ident = constp.tile([128, 128], F32)
            make_identity(nc, ident)

            # resident head weights (wh2/wh3 streamed in the head phase)
            wh1_sb = headwp.tile([P, 4, F1], BF)
            nc.sync.dma_start(out=wh1_sb, in_=wh1[:, :, :])
            wh4_sb = headwp.tile([P, F3 // P, OUT], BF)
            nc.sync.dma_start(out=wh4_sb, in_=wh4[:, :, :])
            bh1_sb = headwp.tile([P, F1 // P], F32)
            nc.sync.dma_start(out=bh1_sb, in_=bh1s[:, :])
            bh2_sb = headwp.tile([P, F2 // P], F32)
            nc.sync.dma_start(out=bh2_sb, in_=bh2s[:, :])
            bh3_sb = headwp.tile([P, F3 // P], F32)
            nc.sync.dma_start(out=bh3_sb, in_=bh3s[:, :])
            bh4_sb = headwp.tile([OUT, 1], F32)
            nc.sync.dma_start(out=bh4_sb, in_=bh4s[:, :])

            # ---- LSTM state ----
            hT = statep.tile([P, KT, NLOC], BF)
            cT = statep.tile([P, KT, NLOC], F32)
            nc.vector.memset(hT, 0.0)
            nc.vector.memset(cT, 0.0)

            # ================= GCN (tiny) =================
            with (
                tc.tile_pool(name="gcn", bufs=2) as gcnp,
                tc.tile_pool(name="gcn1", bufs=1) as gcn1p,
                tc.tile_pool(name="gcn_ps", bufs=2, space="PSUM") as gcnps,
            ):
                xT_sb = gcn1p.tile([L, T], F32)
                for i in range(2):
                    xt = gcnp.tile([96, L], F32, tag="xt")
                    nc.sync.dma_start(out=xt, in_=xb[i])
                    xT_ps = gcnps.tile([L, 96], F32, tag="tp")
                    nc.tensor.transpose(xT_ps, xt, ident[:96, :96])
                    nc.scalar.copy(xT_sb[:, i * 96 : (i + 1) * 96], xT_ps)
                mT_ps = gcnps.tile([L, T], F32, tag="mm")
                nc.tensor.matmul(mT_ps, lhsT=a_sb, rhs=xT_sb, start=True, stop=True)
                mp_sb = gcn1p.tile([L, T], F32)
                mm_sb = gcn1p.tile([L, T], F32)
                nc.scalar.activation(mp_sb, mT_ps, AF.Relu)
                nc.scalar.activation(mm_sb, mT_ps, AF.Relu, scale=-1.0)
                for src, dst in ((mp_sb, p_dram), (mm_sb, q_dram)):
                    rT_ps = gcnps.tile([L, T], F32, tag="mm")
                    nc.tensor.matmul(rT_ps, lhsT=a_sb, rhs=src, start=True, stop=True)
                    rT_sb = gcnp.tile([L, T], F32, tag="rt")
                    nc.scalar.copy(rT_sb, rT_ps)
                    for i in range(2):
                        r_ps = gcnps.tile([96, L], F32, tag="tp2")
                        nc.tensor.transpose(
                            r_ps, rT_sb[:, i * 96 : (i + 1) * 96], ident[:L, :L]
                        )
                        r_sb = gcnp.tile([96, L], BF, tag="rsb")
                        nc.scalar.copy(r_sb, r_ps)
                        nc.sync.dma_start(out=dst[i * 96 : (i + 1) * 96, :], in_=r_sb)

            # ============ production + LSTM ============
            with (
                tc.tile_pool(name="pq", bufs=3) as pqp,
                tc.tile_pool(name="h2", bufs=2) as h2p,
                tc.tile_pool(name="gx", bufs=2) as gxp,
                tc.tile_pool(name="ltmp", bufs=3) as ltp,
                tc.tile_pool(name="h2_ps", bufs=2, space="PSUM") as h2ps,
                tc.tile_pool(name="gx_ps", bufs=2, space="PSUM") as gxps,
                tc.tile_pool(name="rec_ps", bufs=1, space="PSUM") as recps,
            ):
                h2_tiles = [None] * NCH
                gx_tiles = [None] * NCH

                pq_tiles = [None] * NCH

                def produce_pq(c):
                    pq = pqp.tile([2, CH, NLOC], BF, tag="pq", name="pq")
                    nc.sync.dma_start(
                        out=pq[0:1], in_=p_dram[c * CH : (c + 1) * CH, 0:NLOC][None]
                    )
                    nc.sync.dma_start(
                        out=pq[1:2], in_=q_dram[c * CH : (c + 1) * CH, 0:NLOC][None]
                    )
                    pq_tiles[c] = pq

                def produce_h2(c):
                    """h2T = relu(u+ p + u- q) for chunk c."""
                    pq = pq_tiles[c]
                    h2 = h2p.tile([P, KT, NPOS], BF, tag="h2")
                    for gt in range(KT):
                        h2_ps = h2ps.tile([P, NPOS], F32, tag="h2ps")
                        nc.tensor.matmul(
                            h2_ps,
                            lhsT=uu_sb[:, gt * P : (gt + 1) * P],
                            rhs=pq,
                            start=True,
                            stop=True,
                        )
                        nc.scalar.activation(h2[:, gt], h2_ps, AF.Relu)
                    h2_tiles[c] = h2

                def produce_gx_mtile(c, m):
                    """one m-tile of gxT = W_ih @ h2 + bias for chunk c."""
                    if m == 0:
                        gx_tiles[c] = gxp.tile([P, MT, NPOS], BF, tag="gx", name="gx")
                    gx = gx_tiles[c]
                    h2 = h2_tiles[c]
                    g_ps = gxps.tile([P, NPOS], F32, tag="gps")
                    for k in range(KT):
                        nc.tensor.matmul(
                            g_ps,
                            lhsT=wih_sb[:, k, m * P : (m + 1) * P],
                            rhs=h2[:, k],
                            start=(k == 0),
                            stop=(k == KT - 1),
                        )
                    # psum->sbuf copy with bias on the scalar engine (keeps
                    # the vector engine free for the LSTM state updates)
                    nc.scalar.activation(
                        gx[:, m], g_ps, AF.Identity, bias=b16_sb[:, m : m + 1]
                    )

                # prologue: chunk 0 production runs un-overlapped
                produce_pq(0)
                produce_pq(1)
                produce_h2(0)
                for m in range(MT):
                    produce_gx_mtile(0, m)

                def rec_gate(gate, dst_ps):
                    """16 matmul pairs for one gate (4 m-tiles x 4 k)."""
                    for mi in range(4):
                        m = gate * 4 + mi
                        for k in range(KT):
                            nc.tensor.matmul(
                                dst_ps[:, mi, 0:NLOC],
                                lhsT=whh_sb[:, k, m * P : (m + 1) * P],
                                rhs=hT[:, k],
                                start=(k == 0),
                                stop=(k == KT - 1),
                            )

                def gate_total(name, dst, gate, gx, s, ps):
                    gt = ltp.tile([P, 4, NLOC], F32, tag=name, name=name)
                    nc.vector.scalar_tensor_tensor(
                        out=gt,
                        in0=ps[:, 0:4, 0:NLOC],
                        scalar=1.0 / WHH_SCALE,
                        in1=gx[:, gate * 4 : gate * 4 + 4, s * NLOC : (s + 1) * NLOC],
                        op0=ALU.mult,
                        op1=ALU.add,
                    )
                    return gt

                # device gate order: 0=g 1=f 2=i 3=o (one PSUM bank each)
                for c in range(NCH):
                    gx = gx_tiles[c]
                    for s in range(CH):
                        rps = [
                            recps.tile(
                                [P, 4, 64], F32, tag=f"rec{gi}", name=f"rec{gi}"
                            )
                            for gi in range(4)
                        ]
                        # g gate
                        rec_gate(0, rps[0])
                        gt_g = gate_total("gtg", None, 0, gx, s, rps[0])
                        tg = ltp.tile([P, 4, NLOC], F32, tag="tg")
                        nc.scalar.activation(tg, gt_g, AF.Tanh)
                        # f gate
                        rec_gate(1, rps[1])
                        gt_f = gate_total("gtf", None, 1, gx, s, rps[1])
                        sf = ltp.tile([P, 4, NLOC], F32, tag="sf")
                        nc.scalar.activation(sf, gt_f, AF.Sigmoid)
                        t1 = ltp.tile([P, 4, NLOC], F32, tag="t1")
                        nc.vector.tensor_tensor(t1, sf, cT, op=ALU.mult)
                        # i gate
                        rec_gate(2, rps[2])
                        gt_i = gate_total("gti", None, 2, gx, s, rps[2])
                        si = ltp.tile([P, 4, NLOC], F32, tag="si")
                        nc.scalar.activation(si, gt_i, AF.Sigmoid)
                        t2 = ltp.tile([P, 4, NLOC], F32, tag="t2")
                        nc.vector.tensor_tensor(t2, si, tg, op=ALU.mult)
                        # o gate matmuls, then gap-filler, then c/h updates
                        rec_gate(3, rps[3])
                        if c + 1 < NCH:
                            if s == 0:
                                if c + 2 < NCH:
                                    produce_pq(c + 2)
                                produce_h2(c + 1)
                            produce_gx_mtile(c + 1, 2 * s)
                            produce_gx_mtile(c + 1, 2 * s + 1)
                        nc.vector.tensor_add(cT, t1, t2)
                        tc_ = ltp.tile([P, 4, NLOC], F32, tag="tc")
                        nc.scalar.activation(tc_, cT, AF.Tanh)
                        gt_o = gate_total("gto", None, 3, gx, s, rps[3])
                        so = ltp.tile([P, 4, NLOC], F32, tag="so")
                        nc.scalar.activation(so, gt_o, AF.Sigmoid)
                        nc.vector.tensor_tensor(hT, so, tc_, op=ALU.mult)

            # ================= head =================
            with (
                tc.tile_pool(name="hd", bufs=3) as hdp,
                tc.tile_pool(name="hd1", bufs=1) as hd1p,
                tc.tile_pool(name="hd_ps", bufs=4, space="PSUM") as hdps,
            ):
                z1 = hd1p.tile([P, F1 // P, NLOC], BF)
                for m in range(F1 // P):
                    ps = hdps.tile([P, NLOC], F32, tag="zps")
                    for k in range(4):
                        nc.tensor.matmul(
                            ps,
                            lhsT=wh1_sb[:, k, m * P : (m + 1) * P],
                            rhs=hT[:, k],
                            start=(k == 0),
                            stop=(k == 3),
                        )
                    nc.scalar.activation(
                        z1[:, m], ps, AF.Relu, bias=bh1_sb[:, m : m + 1]
                    )
                z2 = hd1p.tile([P, F2 // P, NLOC], BF)
                for m in range(F2 // P):
                    w2t = hdp.tile([P, F1 // P, P], BF, tag="w2t")
                    nc.sync.dma_start(out=w2t, in_=wh2[:, :, m * P : (m + 1) * P])
                    ps = hdps.tile([P, NLOC], F32, tag="zps")
                    for k in range(F1 // P):
                        nc.tensor.matmul(
                            ps,
                            lhsT=w2t[:, k],
                            rhs=z1[:, k],
                            start=(k == 0),
                            stop=(k == F1 // P - 1),
                        )
                    nc.scalar.activation(
                        z2[:, m], ps, AF.Relu, bias=bh2_sb[:, m : m + 1]
                    )
                z3 = hd1p.tile([P, F3 // P, NLOC], BF)
                for m in range(F3 // P):
                    w3t = hdp.tile([P, F2 // P, P], BF, tag="w3t")
                    nc.sync.dma_start(out=w3t, in_=wh3[:, :, m * P : (m + 1) * P])
                    ps = hdps.tile([P, NLOC], F32, tag="zps")
                    for k in range(F2 // P):
                        nc.tensor.matmul(
                            ps,
                            lhsT=w3t[:, k],
                            rhs=z2[:, k],
                            start=(k == 0),
                            stop=(k == F2 // P - 1),
                        )
                    nc.scalar.activation(
                        z3[:, m], ps, AF.Relu, bias=bh3_sb[:, m : m + 1]
                    )
                ps4 = hdps.tile([OUT, NLOC], F32, tag="z4")
                for k in range(F3 // P):
                    nc.tensor.matmul(
                        ps4,
                        lhsT=wh4_sb[:, k],
                        rhs=z3[:, k],
                        start=(k == 0),
                        stop=(k == F3 // P - 1),
                    )
                y_sb = hd1p.tile([OUT, NLOC], F32)
                nc.scalar.activation(y_sb, ps4, AF.Sigmoid, bias=bh4_sb[:, 0:1])
                nc.sync.dma_start(out=out[:, :], in_=y_sb)

    nc.compile()
    return nc


_PROG = None
_LAST_RESULTS = None


def _get_program():
    global _PROG
    if _PROG is None:
        _PROG = _build_program()
    return _PROG


GATE_PERM = (2, 1, 0, 3)  # device gate order [g, f, i, o] from pytorch [i, f, g, o]


def _pad_gates(w, pad_in, pad_unit):
    """(4*HL, K) -> (4*pad_unit, pad_in), gate blocks permuted to GATE_PERM."""
    H4_, K_ = w.shape
    hl = H4_ // 4
    out = np.zeros((4 * pad_unit, pad_in), w.dtype)
    for g in range(4):
        src = GATE_PERM[g]
        out[g * pad_unit : g * pad_unit + hl, :K_] = w[src * hl : (src + 1) * hl]
    return out


def _kstack(wT, p=P):
    """(K, M) -> (p, K//p, M) partition-major for SBUF staging."""
    K_, M_ = wT.shape
    return np.ascontiguousarray(wT.reshape(K_ // p, p, M_).transpose(1, 0, 2))


def _prep(W1, W2, W_ih, W_hh, b_ih, b_hh, Wh1, bh1, Wh2, bh2, Wh3, bh3, Wh4, bh4):
    f = np.float32
    u_plus = np.maximum(W1[0], 0) @ W2  # (G,)
    u_minus = np.maximum(-W1[0], 0) @ W2
    uu = np.zeros((2, GP), f)
    uu[0, :G] = u_plus
    uu[1, :G] = u_minus
    uu = uu.astype(BF16)

    # W_ih: (2000, 500) -> padded (2048, 512) -> T -> (512, 2048)
    wih_p = _pad_gates(W_ih, GP, HLP)  # (2048, 512)
    wih_t = _kstack(np.ascontiguousarray(wih_p.T)).astype(BF16)  # (128,4,2048)
    whh_p = _pad_gates(W_hh, HLP, HLP) * np.float32(WHH_SCALE)  # (2048, 512)
    whh_t = _kstack(np.ascontiguousarray(whh_p.T)).astype(FP8)
    bias = np.zeros(H4P, f)
    bb = (b_ih + b_hh).astype(f)
    for g in range(4):
        src = GATE_PERM[g]
        bias[g * HLP : g * HLP + HL] = bb[src * HL : (src + 1) * HL]
    bias16 = np.ascontiguousarray(bias.reshape(MT, P).T)  # (128,16)

    def pad2(w, r, c):
        o = np.zeros((r, c), f)
        o[: w.shape[0], : w.shape[1]] = w
        return o

    wh1 = _kstack(pad2(Wh1, HLP, F1)).astype(BF16)
    wh2 = _kstack(pad2(Wh2, F1, F2)).astype(BF16)
    wh3 = _kstack(pad2(Wh3, F2, F3)).astype(BF16)
    wh4 = _kstack(pad2(Wh4, F3, OUT)).astype(BF16)
    bh1s = np.ascontiguousarray(pad2(bh1[None], 1, F1)[0].reshape(F1 // P, P).T)
    bh2s = np.ascontiguousarray(pad2(bh2[None], 1, F2)[0].reshape(F2 // P, P).T)
    bh3s = np.ascontiguousarray(pad2(bh3[None], 1, F3)[0].reshape(F3 // P, P).T)
    bh4s = np.ascontiguousarray(bh4.astype(f).reshape(OUT, 1))
    return uu, wih_t, whh_t, bias16, wh1, wh2, wh3, wh4, bh1s, bh2s, bh3s, bh4s


def kernel(
    x,
    A_hat,
    W1,
    W2,
    W_ih,
    W_hh,
    b_ih,
    b_hh,
    Wh1,
    bh1,
    Wh2,
    bh2,
    Wh3,
    bh3,
    Wh4,
    bh4,
):
    f = np.float32
    x = np.asarray(x, f)
    nc = _get_program()
    args = [
        np.asarray(a, f)
        for a in (
            W1,
            W2,
            W_ih,
            W_hh,
            b_ih,
            b_hh,
            Wh1,
            bh1,
            Wh2,
            bh2,
            Wh3,
            bh3,
            Wh4,
            bh4,
        )
    ]
    uu, wih_t, whh_t, bias16, wh1, wh2, wh3, wh4, bh1s, bh2s, bh3s, bh4s = _prep(*args)
    a_hat = np.ascontiguousarray(np.asarray(A_hat, f))

    # odd cores handle lines 60..119: roll lines so theirs sit at 0..59
    # (the GCN is permutation-equivariant when A_hat is permuted to match)
    a_roll = np.ascontiguousarray(np.roll(np.roll(a_hat, -NLOC, 0), -NLOC, 1))
    in_maps = []
    for c in range(NCORES):
        b = c // 2
        if c % 2 == 0:
            xc, ac = x[b], a_hat
        else:
            xc, ac = np.roll(x[b], -NLOC, axis=-1), a_roll
        in_maps.append(
            {
                "xb": np.ascontiguousarray(xc.reshape(2, 96, L)),
                "a_hat": ac,
                "uu": uu,
                "wih_t": wih_t,
                "whh_t": whh_t,
                "bias16": bias16,
                "wh1": wh1,
                "wh2": wh2,
                "wh3": wh3,
                "wh4": wh4,
                "bh1s": bh1s,
                "bh2s": bh2s,
                "bh3s": bh3s,
                "bh4s": bh4s,
            }
        )

    global _LAST_RESULTS
    _LAST_RESULTS = run_bass_kernel_spmd(nc, in_maps, list(range(NCORES)))
    res = _LAST_RESULTS.results
    y = np.zeros((B, OUT, L), f)
    for c in range(NCORES):
        b = c // 2
        l0 = (c % 2) * NLOC
        y[b, :, l0 : l0 + NLOC] = res[c]["out"]
    return y
